# revision 11
# baseline (speedup 1.0000x reference)
"""DiT block kernel for 8 Trainium2 NeuronCores.

Sharding: data-parallel over (batch, seq-half) -> 8 shards, no collectives.
Each core gets x[b] rotated so its 512 query tokens are rows 0:511; K/V are
computed over the full (rotated) 1024-token sequence, so attention needs no
cross-core communication.
"""

import sys

sys.path.insert(0, "/opt/trn_rl_repo")

import numpy as np

import concourse.bass as bass
import concourse.mybir as mybir
from concourse.bass_utils import run_bass_kernel_spmd
from concourse.masks import make_identity
from concourse.tile import TileContext
from concourse.vector_clock import ScopedClock, VectorClock

# ---------------------------------------------------------------------------
# Walrus in this container caps sync-wait commands per CTRL instruction at a
# small number; Tile's stock tail drain collects one wait per live proc and
# trips that cap. Split the final waits across one SP NOP per proc instead.
_orig_drain_and_barrier = TileContext._drain_and_barrier


def _split_drain_and_barrier(self, tick_clock, wait_clock):
    gc_list = list(tick_clock.global_clock)
    for p, tick in enumerate(gc_list):
        if tick > 0:
            partial = [0] * len(gc_list)
            partial[p] = tick
            nop = self.nc.sync.nop()
            wait_clock.add_sem_waits(nop.ins, ScopedClock({None: VectorClock(partial)}))
    drain_inst = self.nc.sync.drain()
    req = ScopedClock({None: tick_clock.global_clock.copy()})
    cur = ScopedClock({None: tick_clock.global_clock.copy()})
    wait_clock.add_sem_waits(drain_inst.ins, req, cur)
    self.nc.all_engine_barrier()
    popped = self.nc._tile_sem_poison_stack.pop()
    assert popped is self._sem_poison
    self.nc.clear_and_free_semaphores(list(self.sems.allocated().values()))
    self.nc.all_engine_barrier()


TileContext._drain_and_barrier = _split_drain_and_barrier

# This walrus also caps waits per *compute/DMA* instruction (the S3_LW struct
# allows a single wait). Intercept every instruction Tile commits to a basic
# block and spill all but the last wait onto preceding same-engine NOPs.
_nop_proto = None


def _get_nop_proto():
    global _nop_proto
    if _nop_proto is None:
        scratch = bass.Bass()
        _nop_proto = scratch.sync.nop().ins
        _nop_proto.sync_info = None
    return _nop_proto


_orig_add_instruction = TileContext._add_instruction


def _add_instruction_capped(self, inst):
    si = inst.sync_info
    if si is not None and si.on_wait is not None and len(si.on_wait) > 1:
        waits = list(si.on_wait)
        si.on_wait = waits[-1:]
        import copy as _copy

        for w in waits[:-1]:
            nop = _copy.deepcopy(_get_nop_proto())
            nop.name = self.nc.get_next_instruction_name()
            nop.engine = inst.engine
            nop.sync_info = mybir.SyncInfo(on_wait=[w], on_update=[])
            _orig_add_instruction(self, nop)
    _orig_add_instruction(self, inst)


TileContext._add_instruction = _add_instruction_capped

# ---------------------------------------------------------------------------

FP32 = mybir.dt.float32
FP32R = mybir.dt.float32r
AF = mybir.ActivationFunctionType
ALU = mybir.AluOpType
AX = mybir.AxisListType

D = 1024
H = 16
HD = 64
S = 1024
SQ = 512  # query tokens per core
MLP = 4096
COND = 128
NT = S // 128  # 8 token tiles
NTQ = SQ // 128  # 4 query token tiles
LN_EPS = 1e-5
N_CORES = 8


def _build_nc():
    nc = bass.Bass()

    xb = nc.dram_tensor("xb", [S, D], FP32, kind="ExternalInput")
    crep = nc.dram_tensor("crep", [COND, 128], FP32R, kind="ExternalInput")
    adaw = nc.dram_tensor("adaw", [COND, 6 * D], FP32R, kind="ExternalInput")
    adab = nc.dram_tensor("adab", [1, 6 * D], FP32R, kind="ExternalInput")
    cvec = nc.dram_tensor("cvec", [1, 3 * D], FP32R, kind="ExternalInput")
    b1c = nc.dram_tensor("b1c", [128, 32], FP32, kind="ExternalInput")
    wqkv = nc.dram_tensor("wqkv", [D, 3 * D], FP32R, kind="ExternalInput")
    wout = nc.dram_tensor("wout", [D, D], FP32R, kind="ExternalInput")
    w1d = nc.dram_tensor("w1", [D, MLP], FP32R, kind="ExternalInput")
    w2d = nc.dram_tensor("w2", [MLP, D], FP32R, kind="ExternalInput")
    cosr = nc.dram_tensor("cosr", [S, 512], FP32, kind="ExternalInput")
    sinr = nc.dram_tensor("sinr", [S, 512], FP32, kind="ExternalInput")
    outd = nc.dram_tensor("out", [SQ, D], FP32, kind="ExternalOutput")

    with TileContext(nc) as tc:
        # ------------------------------------------------------ persistent
        const_cm = tc.tile_pool(name="const", bufs=1)
        const = const_cm.__enter__()
        ident = const.tile([128, 128], FP32, tag="ident")
        make_identity(nc, ident)
        ones128 = const.tile([128, 128], FP32, tag="ones128")
        nc.vector.memset(ones128, 1.0)
        ones_r = const.tile([1, 128], FP32R, tag="ones")
        nc.vector.tensor_copy(ones_r, ones128[0:1, :])
        epst = const.tile([128, 1], FP32, tag="eps")
        nc.vector.memset(epst, LN_EPS)
        b1c_sb = const.tile([128, 32], FP32, tag="b1c")
        nc.sync.dma_start(out=b1c_sb, in_=b1c[:, :])

        vec_cm = tc.tile_pool(name="vec", bufs=1)
        vec = vec_cm.__enter__()
        m1 = vec.tile([128, D], FP32, tag="m1")
        shift_msa = vec.tile([128, D], FP32, tag="shmsa")
        gate_msa = vec.tile([128, D], FP32, tag="gmsa")
        m2 = vec.tile([128, D], FP32, tag="m2")
        shift_mlp = vec.tile([128, D], FP32, tag="shmlp")
        gate_mlp = vec.tile([128, D], FP32, tag="gmlp")
        b2g = vec.tile([128, D], FP32, tag="b2g")

        # ------------------------------------------------ stage 0: adaLN mod
        with (
            tc.tile_pool(name="pre", bufs=1) as pre,
            tc.tile_pool(name="pre_ps", bufs=4, space="PSUM") as pps,
        ):
            adaw_sb = pre.tile([128, 6 * D], FP32R, tag="adaw")
            nc.sync.dma_start(out=adaw_sb, in_=adaw[:, :])
            adab_sb = pre.tile([1, 6 * D], FP32R, tag="adab")
            nc.sync.dma_start(out=adab_sb, in_=adab[:, :])
            cvec_sb = pre.tile([1, 3 * D], FP32R, tag="cvec")
            nc.sync.dma_start(out=cvec_sb, in_=cvec[:, :])
            crep_sb = pre.tile([128, 128], FP32R, tag="crep")
            nc.sync.dma_start(out=crep_sb, in_=crep[:, :])

            modb = pre.tile([128, 6 * D], FP32, tag="modb")
            for j in range(12):
                pt = pps.tile([128, 512], FP32, tag="pp")
                nc.tensor.matmul(
                    pt, crep_sb, adaw_sb[:, j * 512 : (j + 1) * 512],
                    start=True, stop=False,
                )
                nc.tensor.matmul(
                    pt, ones_r, adab_sb[:, j * 512 : (j + 1) * 512],
                    start=False, stop=True,
                )
                nc.vector.tensor_copy(modb[:, j * 512 : (j + 1) * 512], pt)
            cb = pre.tile([128, 3 * D], FP32, tag="cb")
            for j in range(6):
                pt = pps.tile([128, 512], FP32, tag="pp")
                nc.tensor.matmul(
                    pt, ones_r, cvec_sb[:, j * 512 : (j + 1) * 512],
                    start=True, stop=True,
                )
                nc.vector.tensor_copy(cb[:, j * 512 : (j + 1) * 512], pt)

            # mod chunks: shift_msa, scale_msa, gate_msa, shift_mlp, scale_mlp, gate_mlp
            nc.vector.tensor_copy(shift_msa, modb[:, 0:D])
            nc.vector.tensor_copy(gate_msa, modb[:, 2 * D : 3 * D])
            nc.vector.tensor_copy(shift_mlp, modb[:, 3 * D : 4 * D])
            nc.vector.tensor_copy(gate_mlp, modb[:, 5 * D : 6 * D])
            nc.vector.tensor_scalar_add(m1, modb[:, D : 2 * D], 1.0)
            nc.vector.tensor_mul(m1, m1, cb[:, 0:D])
            nc.vector.tensor_scalar_add(m2, modb[:, 4 * D : 5 * D], 1.0)
            nc.vector.tensor_mul(m2, m2, cb[:, D : 2 * D])
            nc.vector.tensor_mul(b2g, gate_mlp, cb[:, 2 * D : 3 * D])

        # ---------------------------------------------- LN + modulate helper
        def ln_mod_transpose(xt, mtile, shtile, stats, scratch, tpp, xmp, dest):
            """LayerNorm(xt)*mtile + shtile, transposed into dest[:, dc, tok128].

            dest is a [128, n_dc, ntok] SBUF AP slice covering this token tile:
            dest[:, j, :] receives block j of the transposed modulated tile.
            """
            red = stats.tile([128, 1], FP32, tag="red")
            nc.vector.reduce_sum(out=red, in_=xt, axis=AX.X)
            mu = stats.tile([128, 1], FP32, tag="mu")
            nc.scalar.mul(mu, red, 1.0 / D)
            sq = scratch.tile([128, D], FP32, tag="sq")
            msq = stats.tile([128, 1], FP32, tag="msq")
            nc.scalar.activation(out=sq, in_=xt, func=AF.Square, accum_out=msq)
            mu2 = stats.tile([128, 1], FP32, tag="mu2")
            nc.vector.tensor_mul(mu2, mu, mu)
            var = stats.tile([128, 1], FP32, tag="var")
            nc.vector.tensor_scalar(
                out=var, in0=msq, scalar1=1.0 / D, scalar2=None, op0=ALU.mult
            )
            nc.vector.tensor_sub(var, var, mu2)
            std = stats.tile([128, 1], FP32, tag="std")
            nc.scalar.activation(out=std, in_=var, func=AF.Sqrt, bias=epst)
            rstd = stats.tile([128, 1], FP32, tag="rstd")
            nc.vector.reciprocal(rstd, std)
            xm = xmp.tile([128, D], FP32, tag="xm")
            nc.vector.tensor_scalar(
                out=xm, in0=xt, scalar1=mu, scalar2=rstd,
                op0=ALU.subtract, op1=ALU.mult,
            )
            nc.vector.tensor_mul(xm, xm, mtile)
            nc.vector.tensor_add(xm, xm, shtile)
            for g in range(2):
                pt = tpp.tile([128, 4, 128], FP32, tag="tp")
                for j in range(4):
                    blk = g * 4 + j
                    nc.tensor.transpose(
                        pt[:, j, :], xm[:, blk * 128 : (blk + 1) * 128], ident
                    )
                nc.vector.tensor_copy(dest[:, g * 4 : (g + 1) * 4, :], pt)

        # ------------------------------------------------ stage 1: LN1 -> xmT
        xmT_cm = tc.tile_pool(name="xmTp", bufs=1)
        xmTp = xmT_cm.__enter__()
        xmT = xmTp.tile([128, 8, S], FP32R, tag="xmT")

        with (
            tc.tile_pool(name="s1x", bufs=2) as s1x,
            tc.tile_pool(name="s1st", bufs=3) as s1st,
            tc.tile_pool(name="s1sq", bufs=2) as s1sq,
            tc.tile_pool(name="s1xm", bufs=2) as s1xm,
            tc.tile_pool(name="s1tp", bufs=2, space="PSUM") as s1tp,
        ):
            for tt in range(NT):
                xt = s1x.tile([128, D], FP32, tag="xt")
                nc.sync.dma_start(out=xt, in_=xb[tt * 128 : (tt + 1) * 128, :])
                ln_mod_transpose(
                    xt, m1, shift_msa, s1st, s1sq, s1tp, s1xm,
                    xmT[:, :, tt * 128 : (tt + 1) * 128],
                )

        # ------------------------------------------------ stage 2: QKV + RoPE
        qkT_cm = tc.tile_pool(name="qkTp", bufs=1, side="right")
        qkTp = qkT_cm.__enter__()
        qT = qkTp.tile([128, 8, SQ], FP32R, tag="qT")
        kT = qkTp.tile([128, 8, S], FP32R, tag="kT")
        v_cm = tc.tile_pool(name="vp", bufs=1, side="right")
        vp = v_cm.__enter__()
        v_ext = vp.tile([128, 8, H, HD + 1], FP32R, tag="vext")
        nc.vector.tensor_copy(
            v_ext[:, :, :, HD : HD + 1],
            ones128[:, 0:128].rearrange("p (a b c) -> p a b c", a=8, b=H),
        )

        def qk_group(wcol0, tts, dest, wpool, cpool, rpool, qps, tpp):
            """Project x @ wqkv[:, wcol0:wcol0+1024], rope, transpose into dest.

            dest: [128, 8, ntok] (qT or kT); tts: token tiles to process.
            """
            wA = wpool.tile([128, 8, 1024], FP32R, tag="wA")
            for kc in range(8):
                nc.sync.dma_start(
                    out=wA[:, kc, :],
                    in_=wqkv[kc * 128 : (kc + 1) * 128, wcol0 : wcol0 + 1024],
                )
            for tt in tts:
                ct = cpool.tile([128, 16, 32], FP32, tag="cosr")
                nc.sync.dma_start(
                    out=ct,
                    in_=cosr[tt * 128 : (tt + 1) * 128, :].rearrange(
                        "p (a b) -> p a b", b=32
                    ),
                )
                st = cpool.tile([128, 16, 32], FP32, tag="sinr")
                nc.sync.dma_start(
                    out=st,
                    in_=sinr[tt * 128 : (tt + 1) * 128, :].rearrange(
                        "p (a b) -> p a b", b=32
                    ),
                )
                for fc in range(2):
                    pt = qps.tile([128, 8, HD], FP32, tag="qkvp")
                    ptf = pt.rearrange("p a b -> p (a b)")
                    for kc in range(8):
                        nc.tensor.matmul(
                            ptf,
                            xmT[:, kc, tt * 128 : (tt + 1) * 128],
                            wA[:, kc, fc * 512 : (fc + 1) * 512],
                            start=(kc == 0),
                            stop=(kc == 7),
                        )
                    ro = rpool.tile([128, 8, HD], FP32, tag="rope")
                    ta = rpool.tile([128, 8, 32], FP32, tag="ta")
                    tb = rpool.tile([128, 8, 32], FP32, tag="tb")
                    c8 = ct[:, 0:8, :]
                    s8 = st[:, 0:8, :]
                    nc.vector.tensor_mul(ta, pt[:, :, 0:32], c8)
                    nc.vector.tensor_mul(tb, pt[:, :, 32:64], s8)
                    nc.vector.tensor_sub(ro[:, :, 0:32], ta, tb)
                    ta2 = rpool.tile([128, 8, 32], FP32, tag="ta2")
                    tb2 = rpool.tile([128, 8, 32], FP32, tag="tb2")
                    nc.vector.tensor_mul(ta2, pt[:, :, 32:64], c8)
                    nc.vector.tensor_mul(tb2, pt[:, :, 0:32], s8)
                    nc.vector.tensor_add(ro[:, :, 32:64], ta2, tb2)
                    rof = ro.rearrange("p a b -> p (a b)")
                    ptp = tpp.tile([128, 4, 128], FP32, tag="tp2")
                    for j in range(4):
                        nc.tensor.transpose(
                            ptp[:, j, :], rof[:, j * 128 : (j + 1) * 128], ident
                        )
                    nc.vector.tensor_copy(
                        dest[:, fc * 4 : (fc + 1) * 4, tt * 128 : (tt + 1) * 128], ptp
                    )

        with (
            tc.tile_pool(name="s2c", bufs=2) as s2c,
            tc.tile_pool(name="s2r", bufs=2) as s2r,
            tc.tile_pool(name="s2ps", bufs=3, space="PSUM") as s2ps,
            tc.tile_pool(name="s2tp", bufs=2, space="PSUM") as s2tp,
        ):
            with tc.tile_pool(name="s2wq", bufs=1) as s2wq:
                qk_group(0, range(NTQ), qT, s2wq, s2c, s2r, s2ps, s2tp)
            with tc.tile_pool(name="s2wk", bufs=1) as s2wk:
                qk_group(D, range(NT), kT, s2wk, s2c, s2r, s2ps, s2tp)
            with tc.tile_pool(name="s2wv", bufs=1) as s2wv:
                wB = s2wv.tile([128, 8, 1024], FP32R, tag="wB")
                for kc in range(8):
                    nc.sync.dma_start(
                        out=wB[:, kc, :],
                        in_=wqkv[kc * 128 : (kc + 1) * 128, 2 * D : 3 * D],
                    )
                for tt in range(NT):
                    for fc in range(2):
                        pt = s2ps.tile([128, 8, HD], FP32, tag="qkvp")
                        ptf = pt.rearrange("p a b -> p (a b)")
                        for kc in range(8):
                            nc.tensor.matmul(
                                ptf,
                                xmT[:, kc, tt * 128 : (tt + 1) * 128],
                                wB[:, kc, fc * 512 : (fc + 1) * 512],
                                start=(kc == 0),
                                stop=(kc == 7),
                            )
                        nc.vector.tensor_copy(
                            v_ext[:, tt, fc * 8 : (fc + 1) * 8, 0:HD], pt
                        )

        xmT_cm.__exit__(None, None, None)

        # ------------------------------------------------ stage 3: attention
        attnT_cm = tc.tile_pool(name="attnTp", bufs=1)
        attnTp = attnT_cm.__enter__()
        attnT = attnTp.tile([128, 8, SQ], FP32R, tag="attnT")

        with (
            tc.tile_pool(name="s3st", bufs=3, space="PSUM") as s3st,
            tc.tile_pool(name="s3pv", bufs=2, space="PSUM") as s3pv,
            tc.tile_pool(name="s3bc", bufs=2, space="PSUM") as s3bc,
            tc.tile_pool(name="s3pr", bufs=3) as s3pr,
            tc.tile_pool(name="s3re", bufs=2) as s3re,
        ):
            for h in range(H):
                r0 = (h % 2) * 64
                dc = h // 2
                pv = s3pv.tile([HD + 1, 512], FP32, tag="pv")
                for tt in range(NT):
                    stp = s3st.tile([128, 512], FP32, tag="st")
                    nc.tensor.matmul(
                        stp,
                        kT[r0 : r0 + 64, dc, tt * 128 : (tt + 1) * 128],
                        qT[r0 : r0 + 64, dc, :],
                        start=True,
                        stop=True,
                    )
                    pr = s3pr.tile([128, 512], FP32R, tag="pr")
                    nc.scalar.activation(out=pr, in_=stp, func=AF.Exp, scale=0.125)
                    nc.tensor.matmul(
                        pv,
                        v_ext[:, tt, h, :],
                        pr,
                        start=(tt == 0),
                        stop=(tt == NT - 1),
                    )
                rec = s3re.tile([1, 512], FP32R, tag="rec")
                with nc.allow_low_precision(reason="softmax denom reciprocal in fp32r"):
                    nc.vector.reciprocal(rec, pv[HD : HD + 1, :])
                bc = s3bc.tile([64, 512], FP32, tag="bc")
                nc.tensor.matmul(
                    bc, ones_r[:, 0:64], rec, start=True, stop=True
                )
                bcs = s3re.tile([64, 512], FP32, tag="bcs")
                nc.vector.tensor_copy(bcs, bc)
                nc.vector.tensor_mul(attnT[r0 : r0 + 64, dc, :], pv[0:HD, :], bcs)

        v_cm.__exit__(None, None, None)
        qkT_cm.__exit__(None, None, None)

        # ------------------------------------- stage 4: out-proj + residual 1
        x1_cm = tc.tile_pool(name="x1p", bufs=1, side="right")
        x1p = x1_cm.__enter__()
        x1 = x1p.tile([128, NTQ, D], FP32, tag="x1")

        with (
            tc.tile_pool(name="s4wo", bufs=1) as s4wo,
            tc.tile_pool(name="s4xs", bufs=2) as s4xs,
            tc.tile_pool(name="s4tmp", bufs=2) as s4tmp,
            tc.tile_pool(name="s4ps", bufs=3, space="PSUM") as s4ps,
        ):
            wo = s4wo.tile([128, 8, D], FP32R, tag="wo")
            for kc in range(8):
                nc.sync.dma_start(
                    out=wo[:, kc, :], in_=wout[kc * 128 : (kc + 1) * 128, :]
                )
            for i in range(NTQ):
                xs = s4xs.tile([128, D], FP32, tag="xs")
                nc.sync.dma_start(out=xs, in_=xb[i * 128 : (i + 1) * 128, :])
                for oc in range(2):
                    pt = s4ps.tile([128, 512], FP32, tag="op")
                    for j in range(8):
                        nc.tensor.matmul(
                            pt,
                            attnT[:, j, i * 128 : (i + 1) * 128],
                            wo[:, j, oc * 512 : (oc + 1) * 512],
                            start=(j == 0),
                            stop=(j == 7),
                        )
                    tmp = s4tmp.tile([128, 512], FP32, tag="optmp")
                    nc.vector.tensor_mul(tmp, pt, gate_msa[:, oc * 512 : (oc + 1) * 512])
                    nc.vector.tensor_add(
                        x1[:, i, oc * 512 : (oc + 1) * 512],
                        xs[:, oc * 512 : (oc + 1) * 512],
                        tmp,
                    )

        attnT_cm.__exit__(None, None, None)

        # ------------------------------------------------ stage 5: LN2 -> xm2T
        xm2T_cm = tc.tile_pool(name="xm2Tp", bufs=1)
        xm2Tp = xm2T_cm.__enter__()
        xm2T = xm2Tp.tile([128, 8, SQ], FP32R, tag="xm2T")

        with (
            tc.tile_pool(name="s5st", bufs=3) as s5st,
            tc.tile_pool(name="s5sq", bufs=2) as s5sq,
            tc.tile_pool(name="s5xm", bufs=2) as s5xm,
            tc.tile_pool(name="s5tp", bufs=2, space="PSUM") as s5tp,
        ):
            for i in range(NTQ):
                ln_mod_transpose(
                    x1[:, i, :], m2, shift_mlp, s5st, s5sq, s5tp, s5xm,
                    xm2T[:, :, i * 128 : (i + 1) * 128],
                )
            # fold b2*gate_mlp into the residual: x1 += b2g
            for i in range(NTQ):
                nc.vector.tensor_add(x1[:, i, :], x1[:, i, :], b2g)

        # ------------------------------------------------ stage 6: MLP
        with (
            tc.tile_pool(name="s6h", bufs=1) as s6h,
            tc.tile_pool(name="s6w1", bufs=2) as s6w1,
            tc.tile_pool(name="s6ps", bufs=2, space="PSUM") as s6ps,
        ):
            hT = s6h.tile([128, 32, SQ], FP32R, tag="hT")
            for mg in range(8):
                wst = s6w1.tile([128, 8, 512], FP32R, tag="w1s")
                nc.sync.dma_start(
                    out=wst,
                    in_=w1d[:, mg * 512 : (mg + 1) * 512].rearrange(
                        "(a p) m -> p a m", p=128
                    ),
                )
                for mi in range(4):
                    mc = mg * 4 + mi
                    pt = s6ps.tile([128, 512], FP32, tag="fc1")
                    for kc in range(8):
                        nc.tensor.matmul(
                            pt,
                            wst[:, kc, mi * 128 : (mi + 1) * 128],
                            xm2T[:, kc, :],
                            start=(kc == 0),
                            stop=(kc == 7),
                        )
                    nc.scalar.activation(
                        out=hT[:, mc, :],
                        in_=pt,
                        func=AF.Gelu_apprx_tanh,
                        bias=b1c_sb[:, mc : mc + 1],
                    )

            with (
                tc.tile_pool(name="s6w2", bufs=3) as s6w2,
                tc.tile_pool(name="s6ps2", bufs=1, space="PSUM") as s6ps2,
                tc.tile_pool(name="s6o", bufs=3) as s6o,
            ):
                for oc in range(2):
                    pts = [
                        s6ps2.tile(
                            [128, 512], FP32, tag=f"fc2_{i}", name=f"fc2_{i}"
                        )
                        for i in range(NTQ)
                    ]
                    for fc in range(32):
                        w2b = s6w2.tile([128, 512], FP32R, tag="w2b")
                        nc.sync.dma_start(
                            out=w2b,
                            in_=w2d[fc * 128 : (fc + 1) * 128, oc * 512 : (oc + 1) * 512],
                        )
                        for i in range(NTQ):
                            nc.tensor.matmul(
                                pts[i],
                                hT[:, fc, i * 128 : (i + 1) * 128],
                                w2b,
                                start=(fc == 0),
                                stop=(fc == 31),
                            )
                    for i in range(NTQ):
                        ot = s6o.tile([128, 512], FP32, tag="outs")
                        nc.vector.tensor_mul(
                            ot, pts[i], gate_mlp[:, oc * 512 : (oc + 1) * 512]
                        )
                        nc.vector.tensor_add(
                            ot, ot, x1[:, i, oc * 512 : (oc + 1) * 512]
                        )
                        nc.sync.dma_start(
                            out=outd[i * 128 : (i + 1) * 128, oc * 512 : (oc + 1) * 512],
                            in_=ot,
                        )

        xm2T_cm.__exit__(None, None, None)
        x1_cm.__exit__(None, None, None)
        vec_cm.__exit__(None, None, None)
        const_cm.__exit__(None, None, None)

    return nc


_NC_CACHE = None


def _get_nc():
    global _NC_CACHE
    if _NC_CACHE is None:
        _NC_CACHE = _build_nc()
    return _NC_CACHE


def _make_in_maps(x, c, norm1_w, norm2_w, w_qkv, w_out, w1, b1, w2, b2,
                  adaLN_w, adaLN_b, cos, sin):
    f32 = lambda a: np.ascontiguousarray(np.asarray(a), dtype=np.float32)
    x = f32(x); c = f32(c)
    cvec = np.concatenate([f32(norm1_w), f32(norm2_w), f32(b2)])[None, :]
    b1c = np.ascontiguousarray(f32(b1).reshape(32, 128).T)
    adab = f32(adaLN_b)[None, :]
    cos_rep = np.tile(f32(cos), (1, 16))  # [S, 512]
    sin_rep = np.tile(f32(sin), (1, 16))
    shared = {
        "adaw": f32(adaLN_w), "adab": adab, "cvec": np.ascontiguousarray(cvec),
        "b1c": b1c, "wqkv": f32(w_qkv), "wout": f32(w_out),
        "w1": f32(w1), "w2": f32(w2),
    }
    in_maps = []
    for core in range(N_CORES):
        b, half = core // 2, core % 2
        sh = -half * SQ
        in_maps.append(
            dict(
                shared,
                xb=np.ascontiguousarray(np.roll(x[b], sh, axis=0)),
                crep=np.ascontiguousarray(np.tile(c[b][:, None], (1, 128))),
                cosr=np.ascontiguousarray(np.roll(cos_rep, sh, axis=0)),
                sinr=np.ascontiguousarray(np.roll(sin_rep, sh, axis=0)),
            )
        )
    return in_maps


def _gather(results, x_shape):
    B = x_shape[0]
    out = np.empty(x_shape, dtype=np.float32)
    for core in range(N_CORES):
        b, half = core // 2, core % 2
        out[b, half * SQ : (half + 1) * SQ] = results[core]["out"]
    return out


def run(inputs, trace=False):
    nc = _get_nc()
    in_maps = _make_in_maps(**inputs)
    res = run_bass_kernel_spmd(nc, in_maps, list(range(N_CORES)), trace=trace)
    out = _gather(res.results, np.asarray(inputs["x"]).shape)
    return out, res


def kernel(**inputs):
    out, _ = run(inputs)
    return out
